# revision 15
# baseline (speedup 1.0000x reference)
# Trainium2 Bass kernel for nn_DTEnergy1D (deep-thinking energy CNN).
# Data-parallel over batch: 32 = 8 cores x 4. All convs as shifted matmuls:
# op conv + h1 in float32r (fp22 mantissa, full PE rate), h2 in bf16 (a1
# activations + h2 weights bf16: same PE rate, 2x faster weight loads + half
# the SBUF). LayerNorm via moment reduction; energy-block update folded as
# 0.8*prelu(LN(it),1.25) + conv(it)*(-0.1) + opxc where opxc = -0.1*conv_x(x)
# is precomputed once (x is constant across the recurrence) and stored fp8.
#
# Matmul loops are weight-major with b innermost so one stationary weight
# load serves 4 matmuls (op conv + h1 share a 4-bank PSUM pool).
#
# fp32r matmul ISA restrictions (walrus s3d3_mm_fp32r): dst must be 8B-aligned,
# start at partition 0, with even element counts. So activations that feed
# convs live in a padded layout [P, G, 514] with zero columns at 0 and 513;
# the k-tap shift is taken on the (unaligned-ok) rhs side: rhs = t[:, ci,
# k:k+512], dst always [*, 0:512].
import sys

sys.path.insert(0, '/opt/trn_rl_repo')
import numpy as np
import ml_dtypes

N_CORES = 8
B = 4          # batch shard per core
L = 512
PADL = L + 2   # padded activation row: [0]=0, [1:513]=data, [513]=0
W = 384
EPS = 1e-5

LAST_RESULT = None  # BassKernelResults of the most recent run (for test harness)
USE_PRELU = True    # fused Prelu combine on HW; CoreSim lacks Prelu -> set False


def _build(iters):
    from contextlib import ExitStack
    import concourse.bacc as bacc
    import concourse.mybir as mybir
    import concourse.tile as tile

    F32 = mybir.dt.float32
    F32R = mybir.dt.float32r
    F8 = mybir.dt.float8e4
    BF16 = mybir.dt.bfloat16
    AT = mybir.ActivationFunctionType
    OP = mybir.AluOpType
    D = slice(1, 513)  # data columns within a padded row

    nc = bacc.Bacc("TRN2", target_bir_lowering=False, debug=False,
                   num_devices=N_CORES)

    def f32(ap):
        return ap.bitcast(F32)

    x3_d = nc.dram_tensor("x3", [B, 3, L], F32R, kind="ExternalInput")
    opw_d = nc.dram_tensor("opw", [128, 9, W], BF16, kind="ExternalInput")
    opxw_d = nc.dram_tensor("opxw", [4, W], F32R, kind="ExternalInput")
    projw_d = nc.dram_tensor("projw", [128, W], F32R, kind="ExternalInput")
    h1w_d = nc.dram_tensor("h1w", [128, 9, W], BF16, kind="ExternalInput")
    h2w_d = nc.dram_tensor("h2w", [128, 9, 192], BF16, kind="ExternalInput")
    h3aw_d = nc.dram_tensor("h3aw", [128, 3, 2], F32R, kind="ExternalInput")
    h3bw_d = nc.dram_tensor("h3bw", [128, 2, 2], F32R, kind="ExternalInput")
    ones_d = nc.dram_tensor("ones", [128, 128], F32R, kind="ExternalInput")
    out_d = nc.dram_tensor("out", [B, iters, 2, L], F32, kind="ExternalOutput")

    with tile.TileContext(nc) as tc, ExitStack() as ctx:
        wpool = ctx.enter_context(tc.tile_pool(name="w", bufs=1))
        itpool = ctx.enter_context(tc.tile_pool(name="it", bufs=2))
        a1pool = ctx.enter_context(tc.tile_pool(name="a1", bufs=1))
        a2pool = ctx.enter_context(tc.tile_pool(name="a2", bufs=1))
        scrpool = ctx.enter_context(tc.tile_pool(name="scr", bufs=2))
        stpool = ctx.enter_context(tc.tile_pool(name="st", bufs=2))
        # op conv and h1 share this 4-bank pool (weight-major, b-inner loops)
        ps_mm = ctx.enter_context(tc.tile_pool(name="psmm", bufs=5, space="PSUM"))
        ps_h2 = ctx.enter_context(tc.tile_pool(name="psh2", bufs=2, space="PSUM"))
        ps_misc = ctx.enter_context(tc.tile_pool(name="psmisc", bufs=1, space="PSUM"))

        opw = wpool.tile([128, 9, W], BF16)
        h1w = wpool.tile([128, 9, W], BF16)
        h2w = wpool.tile([128, 9, 192], BF16)
        h3aw = wpool.tile([128, 3, 2], F32R)
        h3bw = wpool.tile([128, 2, 2], F32R)
        opxw = wpool.tile([4, W], F32R)
        projw = wpool.tile([128, W], F32R)
        ones = wpool.tile([128, 128], F32R)
        # zsrc: memset-able f32 zero source; DVE copies produce legal f32r zeros
        zsrc = wpool.tile([128, L], F32, name="zsrc")
        nc.vector.memset(zsrc[:], 0.0)
        x3 = []
        for b in range(B):
            # rows 0:3 = tap-shifted x, row 3 = zero pad (even K for fp32r mm)
            x3t = wpool.tile([4, L], F32R, name=f"x3_{b}")
            nc.vector.tensor_copy(x3t[:], zsrc[0:4, 0:L])
            nc.sync.dma_start(x3t[0:3, :], x3_d[b])
            x3.append(x3t)
        nc.sync.dma_start(projw[:], projw_d[:])
        nc.sync.dma_start(opxw[:], opxw_d[:])
        nc.sync.dma_start(opw[:, 0:5, :], opw_d[:, 0:5, :])
        nc.scalar.dma_start(opw[:, 5:9, :], opw_d[:, 5:9, :])

        # Persistent activation tiles (allocated once, rewritten in place each
        # iteration). Pad columns are zeroed once here; data writes never
        # touch them, so conv-rhs pad reads stay zero with proper deps.
        # partials[:, 4b+co] accumulates per-row sums of IT via the combine's
        # accum_out; cols 4b+3 stay zero.
        it_cur, it_nxt, a1s, a2as, a2bs, opxcs = [], [], [], [], [], []
        partials = wpool.tile([128, 4 * B], F32R, name="partials")
        nc.vector.tensor_copy(partials[:], zsrc[:, 0:4 * B])
        for b in range(B):
            ita = itpool.tile([128, 3, PADL], BF16, tag=f"itA{b}",
                              name=f"itA{b}")
            itb = itpool.tile([128, 3, PADL], BF16, tag=f"itB{b}",
                              name=f"itB{b}")
            a1 = a1pool.tile([128, 3, PADL], BF16, tag=f"a1_{b}",
                             name=f"a1_{b}")
            a2a = a2pool.tile([128, PADL], F32R, tag=f"a2a_{b}",
                              name=f"a2a_{b}")
            a2b = a2pool.tile([128, PADL], F32R, tag=f"a2b_{b}",
                              name=f"a2b_{b}")
            opxc = wpool.tile([128, 3, L], BF16, name=f"opxc_{b}")
            zp = zsrc[:, 0:6].rearrange("p (g c) -> p g c", g=3)
            for tl in (ita, itb, a1):
                nc.vector.tensor_copy(tl[:, :, 0:PADL:PADL - 1], zp)
            nc.vector.tensor_copy(a2a[:, 0:PADL:PADL - 1], zsrc[:, 0:2])
            nc.vector.tensor_copy(a2b[:, 0:PADL:PADL - 1], zsrc[:, 0:2])
            it_cur.append(ita)
            it_nxt.append(itb)
            a1s.append(a1)
            a2as.append(a2a)
            a2bs.append(a2b)
            opxcs.append(opxc)

        nc.scalar.dma_start(h1w[:], h1w_d[:])
        nc.scalar.dma_start(h2w[:], h2w_d[:])
        nc.scalar.dma_start(h3aw[:], h3aw_d[:])
        nc.scalar.dma_start(h3bw[:], h3bw_d[:])
        nc.scalar.dma_start(ones[:], ones_d[:])

        # initial thought: relu(conv(x, proj_w)); seed partials with row sums.
        # opxc[b][:, co, :] = -0.1 * conv_x(x) for output block co (constant
        # across the recurrence; stored fp8, magnitude ~0.03 so quant error
        # is negligible).
        X = mybir.AxisListType.X
        for b in range(B):
            itt = it_cur[b]
            for co in range(3):
                ps = ps_mm.tile([128, L], F32, tag="ps", name=f"ps_proj{b}_{co}")
                nc.tensor.matmul(ps[:], projw[0:4, co * 128:(co + 1) * 128],
                                 x3[b][:])
                nc.scalar.activation(itt[:, co, D], ps[:], AT.Relu)
                psx = ps_mm.tile([128, L], F32, tag="ps",
                                 name=f"ps_opx{b}_{co}")
                nc.tensor.matmul(psx[:], opxw[0:4, co * 128:(co + 1) * 128],
                                 x3[b][:])
                nc.vector.tensor_copy(opxcs[b][:, co, :], psx[:])
            with nc.allow_low_precision("fp32r row sums feed fp32r matmul"):
                for co in range(3):
                    nc.vector.tensor_reduce(partials[:, 4 * b + co:4 * b + co + 1],
                                            itt[:, co, :], X, OP.add)

        inv_n = 1.0 / float(W * L)

        for t in range(iters):
            # ---- LN stats: row sums came free from last iter's combines ----
            ssq = stpool.tile([128, B], F32R, tag="ssq", name=f"ssq{t}")
            with nc.allow_low_precision("fp32r stats feed fp32r matmul reduce"):
                for b in range(B):
                    scr = scrpool.tile([128, 3, PADL], F32, tag="scr",
                                       bufs=1, name=f"scr{t}_{b}")
                    nc.scalar.activation(scr[:], it_cur[b][:], AT.Square,
                                         accum_out=ssq[:, b:b + 1])
            # ps_stats[:, b, :] = partition-reduced row sums of IT_t for b;
            # ps_stats[:, 4, :] = per-b total sumsq
            ps_stats = ps_misc.tile([128, 6, 4], F32, tag="ps_misc",
                                    name=f"ps_stats{t}")
            nc.tensor.matmul(ps_stats[:, 0:B, :], ones[:], partials[:],
                             skip_group_check=True)
            nc.tensor.matmul(ps_stats[:, B, :], ones[:], ssq[:],
                             skip_group_check=True)
            musum = stpool.tile([128, B], F32, tag="musum", name=f"musum{t}")
            mu = stpool.tile([128, B], F32, tag="mu", name=f"mu{t}")
            msq = stpool.tile([128, B], F32, tag="msq", name=f"msq{t}")
            vps = stpool.tile([128, B], F32, tag="vps", name=f"vps{t}")
            var = stpool.tile([128, B], F32, tag="var", name=f"var{t}")
            sd = stpool.tile([128, B], F32, tag="sd", name=f"sd{t}")
            rs08 = stpool.tile([128, B], F32, tag="rs08", name=f"rs08{t}")
            nmrs08 = stpool.tile([128, B], F32, tag="nmrs08", name=f"nmrs08{t}")
            nc.vector.tensor_reduce(musum[:], ps_stats[:, 0:B, :],
                                    mybir.AxisListType.X, OP.add)
            nc.vector.tensor_scalar_mul(mu[:], musum[:], inv_n)
            # All scaled by 1/0.64 so sqrt gives sd/0.8 directly:
            # vps = (sumsq/n + eps)/0.64; msq = mu^2/0.64; var = vps - msq
            nc.vector.tensor_scalar(vps[:], ps_stats[:, B, :], inv_n / 0.64,
                                    EPS / 0.64, OP.mult, OP.add)
            nc.vector.scalar_tensor_tensor(msq[:], mu[:], 1.0 / 0.64, mu[:],
                                           OP.mult, OP.mult)
            nc.vector.tensor_sub(var[:], vps[:], msq[:])
            nc.scalar.activation(sd[:], var[:], AT.Sqrt)
            # rs08 = 0.8/sd; nmrs08 = -mu*rs08  (0.8 folded into prelu scale)
            nc.vector.reciprocal(rs08[:], sd[:])
            nc.vector.scalar_tensor_tensor(nmrs08[:], mu[:], -1.0, rs08[:],
                                           OP.mult, OP.mult)

            # ---- energy block: IT_next = 0.8*prelu(LN) - 0.1*conv + opxc ----
            it_new = it_nxt
            for co in range(3):
                cs = slice(co * 128, (co + 1) * 128)
                pss = [ps_mm.tile([128, L], F32, tag="ps",
                                  name=f"ps_op{t}_{co}_{b}") for b in range(B)]
                for j in range(9):
                    ci, tap = divmod(j, 3)
                    w_ap = opw[:, j, cs]
                    for b in range(B):
                        nc.tensor.matmul(pss[b][:], w_ap,
                                         it_cur[b][:, ci, tap:tap + L],
                                         start=(j == 0), stop=(j == 8))
                for b in range(B):
                    acc = partials[:, 4 * b + co:4 * b + co + 1]
                    lr08 = scrpool.tile([128, L], F32, tag="xl",
                                        bufs=4, name=f"lr{t}_{b}_{co}")
                    nc.scalar.activation(lr08[:], it_cur[b][:, co, D],
                                         AT.Prelu, bias=nmrs08[:, b:b + 1],
                                         scale=rs08[:, b:b + 1], alpha=1.25)
                    lx = scrpool.tile([128, L], F32, tag="lx",
                                      bufs=4, name=f"lx{t}_{b}_{co}")
                    nc.vector.tensor_tensor(lx[:], lr08[:],
                                            opxcs[b][:, co, :], OP.add)
                    nc.vector.scalar_tensor_tensor(
                        it_new[b][:, co, D], pss[b][:], 1.0, lx[:],
                        OP.mult, OP.add, accum_out=acc)

            # ---- output head: h3(relu(h2(relu(h1(new_it))))) ----
            for co in range(3):
                cs = slice(co * 128, (co + 1) * 128)
                pss = [ps_mm.tile([128, L], F32, tag="ps",
                                  name=f"ps_h1{t}_{co}_{b}") for b in range(B)]
                for j in range(9):
                    ci, tap = divmod(j, 3)
                    w_ap = h1w[:, j, cs]
                    for b in range(B):
                        nc.tensor.matmul(pss[b][:], w_ap,
                                         it_new[b][:, ci, tap:tap + L],
                                         start=(j == 0), stop=(j == 8))
                for b in range(B):
                    nc.scalar.activation(a1s[b][:, co, D], pss[b][:], AT.Relu)

            for b in range(B):
                a1 = a1s[b]
                a2a = a2as[b]
                a2b = a2bs[b]
                ps2a = ps_h2.tile([128, L], F32, tag="ps_h2",
                                  name=f"ps2a{t}_{b}")
                i = 0
                for ci in range(3):
                    for tap in range(3):
                        i += 1
                        nc.tensor.matmul(
                            ps2a[:], h2w[:, ci * 3 + tap, 0:128],
                            a1[:, ci, tap:tap + L],
                            start=(i == 1), stop=(i == 9))
                nc.scalar.activation(a2a[:, D], ps2a[:], AT.Relu)
                # h2b split over PE col groups: taps 0-4 -> cols 0:64,
                # taps 5-8 -> cols 64:128 (concurrent), summed on DVE
                ps2b = ps_h2.tile([128, L], F32, tag="ps_h2",
                                  name=f"ps2b{t}_{b}")
                for j in (0, 5, 1, 6, 2, 7, 3, 8, 4):
                    ci, tap = divmod(j, 3)
                    if j < 5:
                        dst, tp2 = ps2b[0:64, :], (0, 0)
                        st, sp = (j == 0), (j == 4)
                    else:
                        dst, tp2 = ps2b[64:128, :], (0, 64)
                        st, sp = (j == 5), (j == 8)
                    nc.tensor.matmul(dst, h2w[:, j, 128:192],
                                     a1[:, ci, tap:tap + L],
                                     start=st, stop=sp, tile_position=tp2)
                h2sb = scrpool.tile([64, L], F32, tag="h2s",
                                    name=f"h2sb{t}_{b}")
                nc.scalar.copy(h2sb[:], ps2b[64:128, :])
                h2p = scrpool.tile([64, L], F32, tag="h2p",
                                   name=f"h2p{t}_{b}")
                nc.vector.tensor_tensor(h2p[:], ps2b[0:64, :], h2sb[:],
                                        OP.add)
                nc.scalar.activation(a2b[0:64, D], h2p[:], AT.Relu)
                # rows 64:128 = rows 0:64 shifted left one col (tap+1 operand)
                nc.sync.dma_start(a2b[64:128, 0:PADL - 1], a2b[0:64, 1:PADL])

            for b in range(B):
                a2a = a2as[b]
                a2b = a2bs[b]
                # h3: ci0 3 taps (K=128) + ci1 packed taps0/1 (K=128) + tap2 (K=64)
                ps3 = ps_misc.tile([2, L], F32, tag="ps_misc", name=f"ps3{t}_{b}")
                for i, tap in enumerate((0, 1, 2)):
                    nc.tensor.matmul(
                        ps3[:], h3aw[:, tap, :], a2a[:, tap:tap + L],
                        start=(i == 0), stop=False)
                nc.tensor.matmul(ps3[:], h3bw[:, 0, :], a2b[:, 0:L],
                                 start=False, stop=False)
                nc.tensor.matmul(ps3[:], h3bw[0:64, 1, :], a2b[0:64, 2:2 + L],
                                 start=False, stop=True)
                ot = scrpool.tile([2, L], F32, tag="ot", bufs=2,
                                  name=f"ot{t}_{b}")
                nc.scalar.copy(ot[:], ps3[:])
                nc.sync.dma_start(out_d[b, t], ot[:])

            it_cur, it_nxt = it_new, it_cur

    nc.compile()
    return nc


def _prep_inputs(x_np, proj_w, op_w, h1_w, h2_w, h3_w):
    """Host-side weight relayout. Returns dict of shared arrays + per-core x3."""
    f = np.float32
    opw = np.empty((128, 9, W), ml_dtypes.bfloat16)
    h1w = np.empty((128, 9, W), ml_dtypes.bfloat16)
    h2w = np.empty((128, 9, 192), ml_dtypes.bfloat16)
    for ci in range(3):
        for tap in range(3):
            j = ci * 3 + tap
            opw[:, j, :] = -0.1 * op_w[:, ci * 128:(ci + 1) * 128, tap].T
            h1w[:, j, :] = h1_w[:, ci * 128:(ci + 1) * 128, tap].T
            h2w[:, j, :] = h2_w[:, ci * 128:(ci + 1) * 128, tap].T
    h3aw = np.zeros((128, 3, 2), f)
    h3aw[:, :, 0:2] = h3_w[:, 0:128, :].transpose(1, 2, 0)
    h3bw = np.zeros((128, 2, 2), f)
    h3bw[0:64, 0, 0:2] = h3_w[:, 128:192, 0].T
    h3bw[64:128, 0, 0:2] = h3_w[:, 128:192, 1].T
    h3bw[0:64, 1, 0:2] = h3_w[:, 128:192, 2].T
    opxw = np.zeros((4, W), f)
    opxw[0:3] = -0.1 * op_w[:, W, :].T
    projw = np.zeros((128, W), f)
    projw[0:3] = proj_w[:, 0, :].T
    ones = np.ones((128, 128), f)

    BF = x_np.shape[0]
    x3 = np.zeros((BF, 3, L), f)
    x3[:, 0, 1:512] = x_np[:, 0, 0:511]
    x3[:, 1, :] = x_np[:, 0, :]
    x3[:, 2, 0:511] = x_np[:, 0, 1:512]

    shared = dict(opw=opw, opxw=opxw, projw=projw, h1w=h1w, h2w=h2w,
                  h3aw=h3aw, h3bw=h3bw, ones=ones)
    return shared, x3


def kernel(x, iters_to_do, proj_w, op_w, h1_w, h2_w, h3_w):
    global LAST_RESULT
    from concourse.bass_utils import run_bass_kernel_spmd

    x = np.asarray(x, np.float32)
    iters = int(iters_to_do)
    assert x.shape == (N_CORES * B, 1, L), x.shape

    shared, x3 = _prep_inputs(x, np.asarray(proj_w, np.float32),
                              np.asarray(op_w, np.float32),
                              np.asarray(h1_w, np.float32),
                              np.asarray(h2_w, np.float32),
                              np.asarray(h3_w, np.float32))

    nc = _build(iters)
    core_ids = list(range(N_CORES))
    in_maps = [dict(shared, x3=np.ascontiguousarray(x3[c * B:(c + 1) * B]))
               for c in core_ids]
    res = run_bass_kernel_spmd(nc, in_maps, core_ids)
    LAST_RESULT = res
    out = np.concatenate([res.results[c]["out"] for c in core_ids], axis=0)
    return out.astype(np.float32)


# revision 16
# speedup vs baseline: 1.0057x; 1.0057x over previous
# Trainium2 Bass kernel for nn_DTEnergy1D (deep-thinking energy CNN).
# Data-parallel over batch: 32 = 8 cores x 4. All convs as shifted matmuls:
# op conv + h1 in float32r (fp22 mantissa, full PE rate), h2 in bf16 (a1
# activations + h2 weights bf16: same PE rate, 2x faster weight loads + half
# the SBUF). LayerNorm via moment reduction; energy-block update folded as
# 0.8*prelu(LN(it),1.25) + conv(it)*(-0.1) + opxc where opxc = -0.1*conv_x(x)
# is precomputed once (x is constant across the recurrence) and stored fp8.
#
# Matmul loops are weight-major with b innermost so one stationary weight
# load serves 4 matmuls (op conv + h1 share a 4-bank PSUM pool).
#
# fp32r matmul ISA restrictions (walrus s3d3_mm_fp32r): dst must be 8B-aligned,
# start at partition 0, with even element counts. So activations that feed
# convs live in a padded layout [P, G, 514] with zero columns at 0 and 513;
# the k-tap shift is taken on the (unaligned-ok) rhs side: rhs = t[:, ci,
# k:k+512], dst always [*, 0:512].
import sys

sys.path.insert(0, '/opt/trn_rl_repo')
import numpy as np
import ml_dtypes

N_CORES = 8
B = 4          # batch shard per core
L = 512
PADL = L + 2   # padded activation row: [0]=0, [1:513]=data, [513]=0
W = 384
EPS = 1e-5

LAST_RESULT = None  # BassKernelResults of the most recent run (for test harness)
USE_PRELU = True    # fused Prelu combine on HW; CoreSim lacks Prelu -> set False


def _build(iters):
    from contextlib import ExitStack
    import concourse.bacc as bacc
    import concourse.mybir as mybir
    import concourse.tile as tile

    F32 = mybir.dt.float32
    F32R = mybir.dt.float32r
    F8 = mybir.dt.float8e4
    BF16 = mybir.dt.bfloat16
    AT = mybir.ActivationFunctionType
    OP = mybir.AluOpType
    D = slice(1, 513)  # data columns within a padded row

    nc = bacc.Bacc("TRN2", target_bir_lowering=False, debug=False,
                   num_devices=N_CORES)

    def f32(ap):
        return ap.bitcast(F32)

    x3_d = nc.dram_tensor("x3", [B, 3, L], F32R, kind="ExternalInput")
    opw_d = nc.dram_tensor("opw", [128, 9, W], BF16, kind="ExternalInput")
    opxw_d = nc.dram_tensor("opxw", [4, W], F32R, kind="ExternalInput")
    projw_d = nc.dram_tensor("projw", [128, W], F32R, kind="ExternalInput")
    h1w_d = nc.dram_tensor("h1w", [128, 9, W], BF16, kind="ExternalInput")
    h2w_d = nc.dram_tensor("h2w", [128, 9, 192], BF16, kind="ExternalInput")
    h3aw_d = nc.dram_tensor("h3aw", [128, 3, 2], F32R, kind="ExternalInput")
    h3bw_d = nc.dram_tensor("h3bw", [128, 2, 2], F32R, kind="ExternalInput")
    ones_d = nc.dram_tensor("ones", [128, 128], F32R, kind="ExternalInput")
    out_d = nc.dram_tensor("out", [B, iters, 2, L], F32, kind="ExternalOutput")

    with tile.TileContext(nc) as tc, ExitStack() as ctx:
        wpool = ctx.enter_context(tc.tile_pool(name="w", bufs=1))
        itpool = ctx.enter_context(tc.tile_pool(name="it", bufs=2))
        a1pool = ctx.enter_context(tc.tile_pool(name="a1", bufs=1))
        a2pool = ctx.enter_context(tc.tile_pool(name="a2", bufs=1))
        scrpool = ctx.enter_context(tc.tile_pool(name="scr", bufs=2))
        stpool = ctx.enter_context(tc.tile_pool(name="st", bufs=2))
        # op conv and h1 share this 4-bank pool (weight-major, b-inner loops)
        ps_mm = ctx.enter_context(tc.tile_pool(name="psmm", bufs=5, space="PSUM"))
        ps_h2 = ctx.enter_context(tc.tile_pool(name="psh2", bufs=2, space="PSUM"))
        ps_misc = ctx.enter_context(tc.tile_pool(name="psmisc", bufs=1, space="PSUM"))

        opw = wpool.tile([128, 9, W], BF16)
        h1w = wpool.tile([128, 9, W], BF16)
        h2w = wpool.tile([128, 9, 192], BF16)
        h3aw = wpool.tile([128, 3, 2], F32R)
        h3bw = wpool.tile([128, 2, 2], F32R)
        opxw = wpool.tile([4, W], F32R)
        projw = wpool.tile([128, W], F32R)
        ones = wpool.tile([128, 128], F32R)
        # zsrc: memset-able f32 zero source; DVE copies produce legal f32r zeros
        zsrc = wpool.tile([128, L], F32, name="zsrc")
        nc.vector.memset(zsrc[:], 0.0)
        x3 = []
        for b in range(B):
            # rows 0:3 = tap-shifted x, row 3 = zero pad (even K for fp32r mm)
            x3t = wpool.tile([4, L], F32R, name=f"x3_{b}")
            nc.vector.tensor_copy(x3t[:], zsrc[0:4, 0:L])
            nc.sync.dma_start(x3t[0:3, :], x3_d[b])
            x3.append(x3t)
        nc.sync.dma_start(projw[:], projw_d[:])
        nc.sync.dma_start(opxw[:], opxw_d[:])
        nc.sync.dma_start(opw[:, 0:5, :], opw_d[:, 0:5, :])
        nc.scalar.dma_start(opw[:, 5:9, :], opw_d[:, 5:9, :])

        # Persistent activation tiles (allocated once, rewritten in place each
        # iteration). Pad columns are zeroed once here; data writes never
        # touch them, so conv-rhs pad reads stay zero with proper deps.
        # partials[:, 4b+co] accumulates per-row sums of IT via the combine's
        # accum_out; cols 4b+3 stay zero.
        it_cur, it_nxt, a1s, a2as, a2bs, opxcs = [], [], [], [], [], []
        partials = wpool.tile([128, 4 * B], F32R, name="partials")
        nc.vector.tensor_copy(partials[:], zsrc[:, 0:4 * B])
        for b in range(B):
            ita = itpool.tile([128, 3, PADL], BF16, tag=f"itA{b}",
                              name=f"itA{b}")
            itb = itpool.tile([128, 3, PADL], BF16, tag=f"itB{b}",
                              name=f"itB{b}")
            a1 = a1pool.tile([128, 3, PADL], BF16, tag=f"a1_{b}",
                             name=f"a1_{b}")
            a2a = a2pool.tile([128, PADL], F32R, tag=f"a2a_{b}",
                              name=f"a2a_{b}")
            a2b = a2pool.tile([128, PADL], F32R, tag=f"a2b_{b}",
                              name=f"a2b_{b}")
            opxc = wpool.tile([128, 3, L], BF16, name=f"opxc_{b}")
            zp = zsrc[:, 0:6].rearrange("p (g c) -> p g c", g=3)
            for tl in (ita, itb, a1):
                nc.vector.tensor_copy(tl[:, :, 0:PADL:PADL - 1], zp)
            nc.vector.tensor_copy(a2a[:, 0:PADL:PADL - 1], zsrc[:, 0:2])
            nc.vector.tensor_copy(a2b[:, 0:PADL:PADL - 1], zsrc[:, 0:2])
            it_cur.append(ita)
            it_nxt.append(itb)
            a1s.append(a1)
            a2as.append(a2a)
            a2bs.append(a2b)
            opxcs.append(opxc)

        nc.scalar.dma_start(h1w[:], h1w_d[:])
        nc.scalar.dma_start(h2w[:], h2w_d[:])
        nc.scalar.dma_start(h3aw[:], h3aw_d[:])
        nc.scalar.dma_start(h3bw[:], h3bw_d[:])
        nc.scalar.dma_start(ones[:], ones_d[:])

        # initial thought: relu(conv(x, proj_w)); seed partials with row sums.
        # opxc[b][:, co, :] = -0.1 * conv_x(x) for output block co (constant
        # across the recurrence; stored fp8, magnitude ~0.03 so quant error
        # is negligible).
        X = mybir.AxisListType.X
        for b in range(B):
            itt = it_cur[b]
            for co in range(3):
                ps = ps_mm.tile([128, L], F32, tag="ps", name=f"ps_proj{b}_{co}")
                nc.tensor.matmul(ps[:], projw[0:4, co * 128:(co + 1) * 128],
                                 x3[b][:])
                nc.scalar.activation(itt[:, co, D], ps[:], AT.Relu)
                psx = ps_mm.tile([128, L], F32, tag="ps",
                                 name=f"ps_opx{b}_{co}")
                nc.tensor.matmul(psx[:], opxw[0:4, co * 128:(co + 1) * 128],
                                 x3[b][:])
                nc.vector.tensor_copy(opxcs[b][:, co, :], psx[:])
            with nc.allow_low_precision("fp32r row sums feed fp32r matmul"):
                for co in range(3):
                    nc.vector.tensor_reduce(partials[:, 4 * b + co:4 * b + co + 1],
                                            itt[:, co, :], X, OP.add)

        inv_n = 1.0 / float(W * L)

        for t in range(iters):
            # ---- LN stats: row sums came free from last iter's combines ----
            ssq = stpool.tile([128, B], F32R, tag="ssq", name=f"ssq{t}")
            with nc.allow_low_precision("fp32r stats feed fp32r matmul reduce"):
                for b in range(B):
                    scr = scrpool.tile([128, 3, PADL], F32, tag="scr",
                                       bufs=1, name=f"scr{t}_{b}")
                    nc.scalar.activation(scr[:], it_cur[b][:], AT.Square,
                                         accum_out=ssq[:, b:b + 1])
            # ps_stats[:, b, :] = partition-reduced row sums of IT_t for b;
            # ps_stats[:, 4, :] = per-b total sumsq
            ps_stats = ps_misc.tile([128, 6, 4], F32, tag="ps_misc",
                                    name=f"ps_stats{t}")
            nc.tensor.matmul(ps_stats[:, 0:B, :], ones[:], partials[:],
                             skip_group_check=True)
            nc.tensor.matmul(ps_stats[:, B, :], ones[:], ssq[:],
                             skip_group_check=True)
            musum = stpool.tile([128, B], F32, tag="musum", name=f"musum{t}")
            mu = stpool.tile([128, B], F32, tag="mu", name=f"mu{t}")
            msq = stpool.tile([128, B], F32, tag="msq", name=f"msq{t}")
            vps = stpool.tile([128, B], F32, tag="vps", name=f"vps{t}")
            var = stpool.tile([128, B], F32, tag="var", name=f"var{t}")
            sd = stpool.tile([128, B], F32, tag="sd", name=f"sd{t}")
            rs08 = stpool.tile([128, B], F32, tag="rs08", name=f"rs08{t}")
            nmrs08 = stpool.tile([128, B], F32, tag="nmrs08", name=f"nmrs08{t}")
            nc.vector.tensor_reduce(musum[:], ps_stats[:, 0:B, :],
                                    mybir.AxisListType.X, OP.add)
            nc.vector.tensor_scalar_mul(mu[:], musum[:], inv_n)
            # All scaled by 1/0.64 so sqrt gives sd/0.8 directly:
            # vps = (sumsq/n + eps)/0.64; msq = mu^2/0.64; var = vps - msq
            nc.vector.tensor_scalar(vps[:], ps_stats[:, B, :], inv_n / 0.64,
                                    EPS / 0.64, OP.mult, OP.add)
            nc.vector.scalar_tensor_tensor(msq[:], mu[:], 1.0 / 0.64, mu[:],
                                           OP.mult, OP.mult)
            nc.vector.tensor_sub(var[:], vps[:], msq[:])
            nc.scalar.activation(sd[:], var[:], AT.Sqrt)
            # rs08 = 0.8/sd; nmrs08 = -mu*rs08  (0.8 folded into prelu scale)
            nc.vector.reciprocal(rs08[:], sd[:])
            nc.vector.scalar_tensor_tensor(nmrs08[:], mu[:], -1.0, rs08[:],
                                           OP.mult, OP.mult)

            # ---- energy block: IT_next = 0.8*prelu(LN) - 0.1*conv + opxc ----
            it_new = it_nxt
            for co in range(3):
                cs = slice(co * 128, (co + 1) * 128)
                pss = [ps_mm.tile([128, L], F32, tag="ps",
                                  name=f"ps_op{t}_{co}_{b}") for b in range(B)]
                for j in range(9):
                    ci, tap = divmod(j, 3)
                    w_ap = opw[:, j, cs]
                    for b in range(B):
                        nc.tensor.matmul(pss[b][:], w_ap,
                                         it_cur[b][:, ci, tap:tap + L],
                                         start=(j == 0), stop=(j == 8))
                for b in range(B):
                    acc = partials[:, 4 * b + co:4 * b + co + 1]
                    lr08 = scrpool.tile([128, L], F32, tag="xl",
                                        name=f"lr{t}_{b}_{co}")
                    nc.scalar.activation(lr08[:], it_cur[b][:, co, D],
                                         AT.Prelu, bias=nmrs08[:, b:b + 1],
                                         scale=rs08[:, b:b + 1], alpha=1.25)
                    lx = scrpool.tile([128, L], F32, tag="lx",
                                      name=f"lx{t}_{b}_{co}")
                    nc.vector.tensor_tensor(lx[:], lr08[:],
                                            opxcs[b][:, co, :], OP.add)
                    nc.vector.scalar_tensor_tensor(
                        it_new[b][:, co, D], pss[b][:], 1.0, lx[:],
                        OP.mult, OP.add, accum_out=acc)

            # ---- output head: h3(relu(h2(relu(h1(new_it))))) ----
            for co in range(3):
                cs = slice(co * 128, (co + 1) * 128)
                pss = [ps_mm.tile([128, L], F32, tag="ps",
                                  name=f"ps_h1{t}_{co}_{b}") for b in range(B)]
                for j in range(9):
                    ci, tap = divmod(j, 3)
                    w_ap = h1w[:, j, cs]
                    for b in range(B):
                        nc.tensor.matmul(pss[b][:], w_ap,
                                         it_new[b][:, ci, tap:tap + L],
                                         start=(j == 0), stop=(j == 8))
                for b in range(B):
                    nc.scalar.activation(a1s[b][:, co, D], pss[b][:], AT.Relu)

            for b in range(B):
                a1 = a1s[b]
                a2a = a2as[b]
                a2b = a2bs[b]
                ps2a = ps_h2.tile([128, L], F32, tag="ps_h2",
                                  name=f"ps2a{t}_{b}")
                i = 0
                for ci in range(3):
                    for tap in range(3):
                        i += 1
                        nc.tensor.matmul(
                            ps2a[:], h2w[:, ci * 3 + tap, 0:128],
                            a1[:, ci, tap:tap + L],
                            start=(i == 1), stop=(i == 9))
                nc.scalar.activation(a2a[:, D], ps2a[:], AT.Relu)
                # h2b split over PE col groups: taps 0-4 -> cols 0:64,
                # taps 5-8 -> cols 64:128 (concurrent), summed on DVE
                ps2b = ps_h2.tile([128, L], F32, tag="ps_h2",
                                  name=f"ps2b{t}_{b}")
                for j in (0, 5, 1, 6, 2, 7, 3, 8, 4):
                    ci, tap = divmod(j, 3)
                    if j < 5:
                        dst, tp2 = ps2b[0:64, :], (0, 0)
                        st, sp = (j == 0), (j == 4)
                    else:
                        dst, tp2 = ps2b[64:128, :], (0, 64)
                        st, sp = (j == 5), (j == 8)
                    nc.tensor.matmul(dst, h2w[:, j, 128:192],
                                     a1[:, ci, tap:tap + L],
                                     start=st, stop=sp, tile_position=tp2)
                h2sb = scrpool.tile([64, L], F32, tag="h2s",
                                    name=f"h2sb{t}_{b}")
                nc.scalar.copy(h2sb[:], ps2b[64:128, :])
                h2p = scrpool.tile([64, L], F32, tag="h2p",
                                   name=f"h2p{t}_{b}")
                nc.vector.tensor_tensor(h2p[:], ps2b[0:64, :], h2sb[:],
                                        OP.add)
                nc.scalar.activation(a2b[0:64, D], h2p[:], AT.Relu)
                # rows 64:128 = rows 0:64 shifted left one col (tap+1 operand)
                nc.sync.dma_start(a2b[64:128, 0:PADL - 1], a2b[0:64, 1:PADL])

            for b in range(B):
                a2a = a2as[b]
                a2b = a2bs[b]
                # h3: ci0 3 taps (K=128) + ci1 packed taps0/1 (K=128) + tap2 (K=64)
                ps3 = ps_misc.tile([2, L], F32, tag="ps_misc", name=f"ps3{t}_{b}")
                for i, tap in enumerate((0, 1, 2)):
                    nc.tensor.matmul(
                        ps3[:], h3aw[:, tap, :], a2a[:, tap:tap + L],
                        start=(i == 0), stop=False)
                nc.tensor.matmul(ps3[:], h3bw[:, 0, :], a2b[:, 0:L],
                                 start=False, stop=False)
                nc.tensor.matmul(ps3[:], h3bw[0:64, 1, :], a2b[0:64, 2:2 + L],
                                 start=False, stop=True)
                ot = scrpool.tile([2, L], F32, tag="ot", bufs=2,
                                  name=f"ot{t}_{b}")
                nc.scalar.copy(ot[:], ps3[:])
                nc.sync.dma_start(out_d[b, t], ot[:])

            it_cur, it_nxt = it_new, it_cur

    nc.compile()
    return nc


def _prep_inputs(x_np, proj_w, op_w, h1_w, h2_w, h3_w):
    """Host-side weight relayout. Returns dict of shared arrays + per-core x3."""
    f = np.float32
    opw = np.empty((128, 9, W), ml_dtypes.bfloat16)
    h1w = np.empty((128, 9, W), ml_dtypes.bfloat16)
    h2w = np.empty((128, 9, 192), ml_dtypes.bfloat16)
    for ci in range(3):
        for tap in range(3):
            j = ci * 3 + tap
            opw[:, j, :] = -0.1 * op_w[:, ci * 128:(ci + 1) * 128, tap].T
            h1w[:, j, :] = h1_w[:, ci * 128:(ci + 1) * 128, tap].T
            h2w[:, j, :] = h2_w[:, ci * 128:(ci + 1) * 128, tap].T
    h3aw = np.zeros((128, 3, 2), f)
    h3aw[:, :, 0:2] = h3_w[:, 0:128, :].transpose(1, 2, 0)
    h3bw = np.zeros((128, 2, 2), f)
    h3bw[0:64, 0, 0:2] = h3_w[:, 128:192, 0].T
    h3bw[64:128, 0, 0:2] = h3_w[:, 128:192, 1].T
    h3bw[0:64, 1, 0:2] = h3_w[:, 128:192, 2].T
    opxw = np.zeros((4, W), f)
    opxw[0:3] = -0.1 * op_w[:, W, :].T
    projw = np.zeros((128, W), f)
    projw[0:3] = proj_w[:, 0, :].T
    ones = np.ones((128, 128), f)

    BF = x_np.shape[0]
    x3 = np.zeros((BF, 3, L), f)
    x3[:, 0, 1:512] = x_np[:, 0, 0:511]
    x3[:, 1, :] = x_np[:, 0, :]
    x3[:, 2, 0:511] = x_np[:, 0, 1:512]

    shared = dict(opw=opw, opxw=opxw, projw=projw, h1w=h1w, h2w=h2w,
                  h3aw=h3aw, h3bw=h3bw, ones=ones)
    return shared, x3


def kernel(x, iters_to_do, proj_w, op_w, h1_w, h2_w, h3_w):
    global LAST_RESULT
    from concourse.bass_utils import run_bass_kernel_spmd

    x = np.asarray(x, np.float32)
    iters = int(iters_to_do)
    assert x.shape == (N_CORES * B, 1, L), x.shape

    shared, x3 = _prep_inputs(x, np.asarray(proj_w, np.float32),
                              np.asarray(op_w, np.float32),
                              np.asarray(h1_w, np.float32),
                              np.asarray(h2_w, np.float32),
                              np.asarray(h3_w, np.float32))

    nc = _build(iters)
    core_ids = list(range(N_CORES))
    in_maps = [dict(shared, x3=np.ascontiguousarray(x3[c * B:(c + 1) * B]))
               for c in core_ids]
    res = run_bass_kernel_spmd(nc, in_maps, core_ids)
    LAST_RESULT = res
    out = np.concatenate([res.results[c]["out"] for c in core_ids], axis=0)
    return out.astype(np.float32)
